# revision 5
# baseline (speedup 1.0000x reference)
"""GCN (2-layer, Kipf-Welling) forward on 8 Trainium2 NeuronCores.

Strategy (graph/data parallel, per the dest-sharding hint):
  - Nodes are partitioned across the 8 cores by destination (balanced by
    degree). Edges are bucketed by destination; the "halo exchange" of
    source-node features is materialized host-side at input-sharding time:
    each core's input stream contains the (dinv-scaled) source-node feature
    rows for its edges, bucketed by destination and padded to a fixed,
    core-uniform tile schedule.
  - On device, the segment-sum over each destination block is computed on
    the TensorEngine: for each 128-slot message tile, a one-hot selection
    matrix S (built on the VectorEngine from per-slot relative-destination
    ids) is used as matmul weights, accumulating into a per-32-dest-window
    PSUM region. All FLOPs of the reference (aggregation, x@W1, +b1, relu,
    @W2, normalization) run on device.
  - Layer 1 aggregates raw (dinv-scaled) x (128 feats), then applies W1 on
    device per destination block (transpose + matmul), bias/relu epilogue
    produces the dinv-scaled hidden table h1'. The host then expands h1'
    along the same edge streams (layer-2 halo), and a second launch
    aggregates and applies W2 + normalization, yielding the final output.
"""

import math
import os

import numpy as np
import ml_dtypes

import concourse.bacc as bacc
import concourse.bass as bass
import concourse.mybir as mybir
import concourse.tile as tile
from concourse._compat import get_trn_type
from concourse.bass_utils import run_bass_kernel_spmd

P = 128
N_CORES = 8
IN_DIM = 128
HID = 64
WIN = 32          # dests per window (matmul col-tile quantum)
NWIN = 4          # windows per block (4 * 32 = 128 dests = one PSUM block)
f32 = mybir.dt.float32
bf16 = mybir.dt.bfloat16

_COMPILE_CACHE = {}


# ---------------------------------------------------------------------------
# host-side preprocessing
# ---------------------------------------------------------------------------

def _preprocess(edge_index, n_nodes):
    """Global balanced assignment of destinations to (core, block, window,
    slot) cells and per-core edge streams.

    Returns a dict with the per-core slot->src map, slot->rel id, the
    dest assignment arrays, and schedule constants.
    """
    rng = np.random  # determinism: no randomness used
    E = edge_index.shape[1]
    dst = np.asarray(edge_index[0], dtype=np.int64)
    src = np.asarray(edge_index[1], dtype=np.int64)
    # add self loops
    loops = np.arange(n_nodes, dtype=np.int64)
    dst = np.concatenate([dst, loops])
    src = np.concatenate([src, loops])

    deg = np.bincount(dst, minlength=n_nodes).astype(np.float64)
    dinv = np.where(deg > 0, 1.0 / np.sqrt(deg), 0.0).astype(np.float32)

    # --- balanced dest -> (core, block, window, slot) assignment (snake) ---
    NBLK = math.ceil(n_nodes / (N_CORES * P))  # blocks per core
    n_cells = N_CORES * NBLK * NWIN
    order = np.argsort(-deg, kind="stable")  # high degree first
    # snake over cells so each cell's total degree is balanced
    cells_fwd = np.arange(n_cells)
    assign_cell = np.empty(n_nodes, dtype=np.int64)
    assign_slot = np.empty(n_nodes, dtype=np.int64)
    pos = 0
    rnd = 0
    while pos < n_nodes:
        take = min(n_cells, n_nodes - pos)
        cells = cells_fwd if (rnd % 2 == 0) else cells_fwd[::-1]
        assign_cell[order[pos : pos + take]] = cells[:take]
        assign_slot[order[pos : pos + take]] = rnd
        pos += take
        rnd += 1
    assert rnd <= WIN, "more than 32 rounds => cell overflow"

    # node -> (core, block, window, slot)
    node_core = assign_cell // (NBLK * NWIN)
    node_blk = (assign_cell // NWIN) % NBLK
    node_win = assign_cell % NWIN
    node_slot = assign_slot  # 0..31 within window
    # partition index within block = win*32 + slot
    node_part = node_win * WIN + node_slot

    # per-cell edge counts to size the uniform schedule
    edge_cell = assign_cell[dst]
    cell_counts = np.bincount(edge_cell, minlength=n_cells)
    max_cell = int(cell_counts.max())
    TWIN = max(9, math.ceil(max_cell / P))  # tiles per window (uniform)
    TBLK = NWIN * TWIN
    TTOT = NBLK * TBLK

    # order edges by (cell, slot-within-window) so each cell's stream is
    # grouped by destination slot (rel id = slot)
    edge_key = edge_cell * WIN + node_slot[dst]
    eorder = np.argsort(edge_key, kind="stable")
    es_cell = edge_cell[eorder]
    es_rel = node_slot[dst][eorder]
    es_src = src[eorder]

    # slot arrays per core: [TTOT, P] src and rel, padded
    cap = TWIN * P  # slots per cell
    src_map = np.zeros((N_CORES, NBLK, NWIN, cap), dtype=np.int64)
    rel_map = np.full((N_CORES, NBLK, NWIN, cap), 33.0, dtype=np.float32)
    # offsets of each cell's stream within the sorted edge list
    cell_starts = np.zeros(n_cells + 1, dtype=np.int64)
    np.cumsum(cell_counts, out=cell_starts[1:])
    # fill per cell (vectorized over edges)
    within = np.arange(len(es_cell)) - cell_starts[es_cell]
    c_core = es_cell // (NBLK * NWIN)
    c_blk = (es_cell // NWIN) % NBLK
    c_win = es_cell % NWIN
    src_map[c_core, c_blk, c_win, within] = es_src
    rel_map[c_core, c_blk, c_win, within] = es_rel.astype(np.float32)

    # reshape to per-core [P, TTOT]: tile t of block b = window-major
    # cell stream slot s -> tile (s // P), partition (s % P)
    src_map = src_map.reshape(N_CORES, NBLK, NWIN, TWIN, P)
    rel_map = rel_map.reshape(N_CORES, NBLK, NWIN, TWIN, P)
    # -> [core, P, NBLK, NWIN, TWIN] -> [core, P, TTOT]
    src_map = np.transpose(src_map, (0, 4, 1, 2, 3)).reshape(N_CORES, P, TTOT)
    rel_map = np.transpose(rel_map, (0, 4, 1, 2, 3)).reshape(N_CORES, P, TTOT)

    # dest node id per (core, part, blk) (or -1 for unused)
    dest_id = np.full((N_CORES, P, NBLK), -1, dtype=np.int64)
    dest_id[node_core, node_part, node_blk] = np.arange(n_nodes)

    return dict(
        dinv=dinv,
        NBLK=NBLK,
        TWIN=TWIN,
        TBLK=TBLK,
        TTOT=TTOT,
        src_map=src_map,
        rel_map=rel_map,
        dest_id=dest_id,
        node_core=node_core,
        node_part=node_part,
        node_blk=node_blk,
    )


# ---------------------------------------------------------------------------
# device programs
# ---------------------------------------------------------------------------

def _build_l1(NBLK, TWIN, TTOT):
    """Layer 1: aggregate dinv-scaled x (128 f), apply W1, bias+relu, emit
    dinv-scaled hidden table h1p [P, NBLK, HID] bf16."""
    TBLK = NWIN * TWIN
    nc = bacc.Bacc(get_trn_type() or "TRN2", debug=False)
    msgs = nc.dram_tensor("msgs", [P, TTOT, IN_DIM], bf16, kind="ExternalInput")
    rel = nc.dram_tensor("rel", [P, TTOT], bf16, kind="ExternalInput")
    iota = nc.dram_tensor("iota", [P, WIN], bf16, kind="ExternalInput")
    w1 = nc.dram_tensor("w1", [IN_DIM, HID], f32, kind="ExternalInput")
    ident = nc.dram_tensor("ident", [P, P], f32, kind="ExternalInput")
    b1d = nc.dram_tensor("b1d", [P, NBLK, HID], f32, kind="ExternalInput")
    dinv2 = nc.dram_tensor("dinv2", [P, NBLK], f32, kind="ExternalInput")
    h1p = nc.dram_tensor("h1p", [P, NBLK, HID], bf16, kind="ExternalOutput")

    with tile.TileContext(nc) as tc:
        with (
            tc.tile_pool(name="const", bufs=1) as constp,
            tc.tile_pool(name="msg", bufs=3) as msgp,
            tc.tile_pool(name="sbuild", bufs=3) as sp,
            tc.tile_pool(name="aggx", bufs=2) as aggp,
            tc.tile_pool(name="aggxt", bufs=2) as aggtp,
            tc.tile_pool(name="t1", bufs=2) as t1p,
            tc.tile_pool(name="stage", bufs=1) as stagep,
            tc.tile_pool(name="psA", bufs=3, space="PSUM") as psA,
            tc.tile_pool(name="psB", bufs=2, space="PSUM") as psB,
            tc.tile_pool(name="psC", bufs=2, space="PSUM") as psC,
        ):
            relb = constp.tile([P, TTOT], bf16)
            iotab = constp.tile([P, WIN], bf16)
            w1b = constp.tile([IN_DIM, HID], f32)
            identb = constp.tile([P, P], f32)
            b1b = constp.tile([P, NBLK, HID], f32)
            dinv2b = constp.tile([P, NBLK], f32)
            stageb = stagep.tile([P, NBLK, HID], bf16)
            nc.sync.dma_start(relb[:], rel[:])
            nc.sync.dma_start(iotab[:], iota[:])
            nc.sync.dma_start(w1b[:], w1[:])
            nc.sync.dma_start(identb[:], ident[:])
            nc.sync.dma_start(b1b[:], b1d[:])
            nc.sync.dma_start(dinv2b[:], dinv2[:])

            for b in range(NBLK):
                mb = msgp.tile([P, TBLK, IN_DIM], bf16, tag="msg")
                nc.sync.dma_start(mb[:], msgs[:, b * TBLK : (b + 1) * TBLK, :])
                sb = sp.tile([P, TBLK, WIN], bf16, tag="s")
                nc.vector.tensor_tensor(
                    out=sb[:],
                    in0=relb[:, b * TBLK : (b + 1) * TBLK, None].to_broadcast(
                        [P, TBLK, WIN]
                    ),
                    in1=iotab[:, None, :].to_broadcast([P, TBLK, WIN]),
                    op=mybir.AluOpType.is_equal,
                )
                pa = psA.tile([P, IN_DIM], f32, tag="pa")
                for w in range(NWIN):
                    for j in range(TWIN):
                        t = w * TWIN + j
                        nc.tensor.matmul(
                            pa[w * WIN : (w + 1) * WIN, :],
                            sb[:, t, :],
                            mb[:, t, :],
                            start=(j == 0),
                            stop=(j == TWIN - 1),
                            tile_position=(0, w * WIN),
                        )
                ax = aggp.tile([P, IN_DIM], f32, tag="ax")
                nc.vector.tensor_copy(out=ax[:], in_=pa[:])
                pb = psB.tile([P, P], f32, tag="pb")
                nc.tensor.transpose(pb[:], ax[:], identb[:])
                axt = aggtp.tile([P, P], f32, tag="axt")
                nc.vector.tensor_copy(out=axt[:], in_=pb[:])
                pc = psC.tile([P, HID], f32, tag="pc")
                nc.tensor.matmul(
                    pc[:], axt[:], w1b[:], start=True, stop=True
                )
                t1 = t1p.tile([P, HID], f32, tag="t1")
                nc.vector.scalar_tensor_tensor(
                    out=t1[:],
                    in0=pc[:],
                    scalar=dinv2b[:, b : b + 1],
                    in1=b1b[:, b, :],
                    op0=mybir.AluOpType.mult,
                    op1=mybir.AluOpType.add,
                )
                nc.scalar.activation(
                    out=stageb[:, b, :],
                    in_=t1[:],
                    func=mybir.ActivationFunctionType.Relu,
                )
            nc.sync.dma_start(h1p[:], stageb[:])
    nc.compile()
    return nc


def _build_l2(NBLK, TWIN, TTOT):
    """Layer 2: aggregate h1p messages (64 f), dot with W2, scale by dinv,
    add b2, emit out [P, NBLK] f32."""
    TBLK = NWIN * TWIN
    nc = bacc.Bacc(get_trn_type() or "TRN2", debug=False)
    msgs = nc.dram_tensor("msgs", [P, TTOT, HID], bf16, kind="ExternalInput")
    rel = nc.dram_tensor("rel", [P, TTOT], bf16, kind="ExternalInput")
    iota = nc.dram_tensor("iota", [P, WIN], bf16, kind="ExternalInput")
    w2r = nc.dram_tensor("w2r", [P, HID], f32, kind="ExternalInput")
    dinvt = nc.dram_tensor("dinvt", [P, NBLK], f32, kind="ExternalInput")
    b2r = nc.dram_tensor("b2r", [P, 1], f32, kind="ExternalInput")
    outv = nc.dram_tensor("outv", [P, NBLK], f32, kind="ExternalOutput")

    with tile.TileContext(nc) as tc:
        with (
            tc.tile_pool(name="const", bufs=1) as constp,
            tc.tile_pool(name="msg", bufs=3) as msgp,
            tc.tile_pool(name="sbuild", bufs=3) as sp,
            tc.tile_pool(name="ttr", bufs=2) as ttrp,
            tc.tile_pool(name="stage", bufs=1) as stagep,
            tc.tile_pool(name="psD", bufs=3, space="PSUM") as psD,
        ):
            relb = constp.tile([P, TTOT], bf16)
            iotab = constp.tile([P, WIN], bf16)
            w2b = constp.tile([P, HID], f32)
            dinvb = constp.tile([P, NBLK], f32)
            b2b = constp.tile([P, 1], f32)
            stageb = stagep.tile([P, NBLK], f32)
            nc.sync.dma_start(relb[:], rel[:])
            nc.sync.dma_start(iotab[:], iota[:])
            nc.sync.dma_start(w2b[:], w2r[:])
            nc.sync.dma_start(dinvb[:], dinvt[:])
            nc.sync.dma_start(b2b[:], b2r[:])

            for b in range(NBLK):
                mb = msgp.tile([P, TBLK, HID], bf16, tag="msg")
                nc.sync.dma_start(mb[:], msgs[:, b * TBLK : (b + 1) * TBLK, :])
                sb = sp.tile([P, TBLK, WIN], bf16, tag="s")
                nc.vector.tensor_tensor(
                    out=sb[:],
                    in0=relb[:, b * TBLK : (b + 1) * TBLK, None].to_broadcast(
                        [P, TBLK, WIN]
                    ),
                    in1=iotab[:, None, :].to_broadcast([P, TBLK, WIN]),
                    op=mybir.AluOpType.is_equal,
                )
                pd = psD.tile([P, HID], f32, tag="pd")
                for w in range(NWIN):
                    for j in range(TWIN):
                        t = w * TWIN + j
                        nc.tensor.matmul(
                            pd[w * WIN : (w + 1) * WIN, :],
                            sb[:, t, :],
                            mb[:, t, :],
                            start=(j == 0),
                            stop=(j == TWIN - 1),
                            tile_position=(0, w * WIN),
                        )
                scr = ttrp.tile([P, HID], f32, tag="scr")
                acc = ttrp.tile([P, 1], f32, tag="acc")
                nc.vector.tensor_tensor(
                    out=scr[:],
                    in0=pd[:],
                    in1=w2b[:],
                    op=mybir.AluOpType.mult,
                )
                nc.vector.tensor_reduce(
                    out=acc[:],
                    in_=scr[:],
                    axis=mybir.AxisListType.X,
                    op=mybir.AluOpType.add,
                )
                nc.vector.tensor_scalar(
                    out=stageb[:, b : b + 1],
                    in0=acc[:],
                    scalar1=dinvb[:, b : b + 1],
                    scalar2=b2b[:],
                    op0=mybir.AluOpType.mult,
                    op1=mybir.AluOpType.add,
                )
            nc.sync.dma_start(outv[:], stageb[:])
    nc.compile()
    return nc


# ---------------------------------------------------------------------------
# kernel entry
# ---------------------------------------------------------------------------

def kernel(x, W1, b1, W2, b2, edge_index):
    x = np.asarray(x)
    W1 = np.asarray(W1, dtype=np.float32)
    b1 = np.asarray(b1, dtype=np.float32)
    W2 = np.asarray(W2, dtype=np.float32)
    b2 = np.asarray(b2, dtype=np.float32)
    edge_index = np.asarray(edge_index)
    n_nodes = x.shape[0]

    pp = _preprocess(edge_index, n_nodes)
    dinv = pp["dinv"]
    NBLK, TWIN, TTOT = pp["NBLK"], pp["TWIN"], pp["TTOT"]
    src_map = pp["src_map"]  # [cores, P, TTOT]
    rel_map = pp["rel_map"]
    dest_id = pp["dest_id"]  # [cores, P, NBLK]

    key = (n_nodes, NBLK, TWIN, TTOT)
    if key not in _COMPILE_CACHE:
        import time as _time

        _t = _time.time()
        nc1 = _build_l1(NBLK, TWIN, TTOT)
        print(f"[kernel] L1 built in {_time.time()-_t:.1f}s", flush=True)
        _t = _time.time()
        nc2 = _build_l2(NBLK, TWIN, TTOT)
        print(f"[kernel] L2 built in {_time.time()-_t:.1f}s", flush=True)
        _COMPILE_CACHE[key] = (nc1, nc2)
    nc1, nc2 = _COMPILE_CACHE[key]

    # ---- common host-side tensors
    iota_np = np.tile(np.arange(WIN, dtype=np.float32), (P, 1)).astype(
        ml_dtypes.bfloat16
    )
    ident_np = np.eye(P, dtype=np.float32)
    # dest-side quantities per core
    dvalid = dest_id >= 0
    d_safe = np.where(dvalid, dest_id, 0)
    dinv_d = np.where(dvalid, dinv[d_safe], 0.0).astype(np.float32)  # [c,P,NBLK]
    b1d_np = dinv_d[:, :, :, None] * b1[None, None, None, :]  # [c,P,NBLK,HID]
    dinv2_np = (dinv_d * dinv_d).astype(np.float32)

    xs = (x.astype(np.float32) * dinv[:, None]).astype(ml_dtypes.bfloat16)
    msgs1 = xs[src_map]  # [cores, P, TTOT, IN_DIM] bf16
    rel_bf = rel_map.astype(ml_dtypes.bfloat16)

    in_maps1 = []
    for c in range(N_CORES):
        in_maps1.append(
            {
                "msgs": msgs1[c],
                "rel": rel_bf[c],
                "iota": iota_np,
                "w1": W1,
                "ident": ident_np,
                "b1d": b1d_np[c].astype(np.float32),
                "dinv2": dinv2_np[c],
            }
        )
    print("[kernel] launching L1", flush=True)
    res1 = run_bass_kernel_spmd(
        nc1, in_maps1, core_ids=list(range(N_CORES))
    ).results
    print("[kernel] L1 done", flush=True)

    # assemble h1p table [n_nodes, HID] (dinv-scaled hidden)
    h_table = np.zeros((n_nodes, HID), dtype=np.float32)
    nci, npi, nbi = pp["node_core"], pp["node_part"], pp["node_blk"]
    for c in range(N_CORES):
        h1p = np.asarray(res1[c]["h1p"]).astype(np.float32)  # [P, NBLK, HID]
        sel = nci == c
        h_table[sel] = h1p[npi[sel], nbi[sel]]
    h_bf = h_table.astype(ml_dtypes.bfloat16)
    msgs2 = h_bf[src_map]  # [cores, P, TTOT, HID]

    w2r_np = np.tile(W2[:, 0][None, :], (P, 1)).astype(np.float32)
    b2r_np = np.full((P, 1), float(b2[0]), dtype=np.float32)
    in_maps2 = []
    for c in range(N_CORES):
        in_maps2.append(
            {
                "msgs": msgs2[c],
                "rel": rel_bf[c],
                "iota": iota_np,
                "w2r": w2r_np,
                "dinvt": dinv_d[c],
                "b2r": b2r_np,
            }
        )
    print("[kernel] launching L2", flush=True)
    res2 = run_bass_kernel_spmd(
        nc2, in_maps2, core_ids=list(range(N_CORES))
    ).results
    print("[kernel] L2 done", flush=True)

    out = np.zeros(n_nodes, dtype=np.float32)
    for c in range(N_CORES):
        ov = np.asarray(res2[c]["outv"])  # [P, NBLK]
        sel = nci == c
        out[sel] = ov[npi[sel], nbi[sel]]
    return out
